# revision 1
# baseline (speedup 1.0000x reference)
"""Trainium2 Bass kernel v2 for nn_Apply_on_single_area (segment_reduce).

Self-contained: accepts FULL inputs, shards areas across 8 NeuronCores
(pure data parallel), returns FULL [32768] f32 output.

Design (vs v1): the whole pipeline runs in bf16 with fp32 accumulations,
smoothing rounds v' = v + sin(v) are single fused DVE ops evaluating a
degree-7 odd minimax polynomial (max err 2.5e-4), the variance inner
term ((q-mean)P)^2 + its sum is one fused DVE op, per-batch scalar work
is deferred to a single vectorized finish phase, and inputs ship as bf16
(masks pre-centered by -0.5 on host for fine quantization near the
sensitive 0.5 fixed point).  End-to-end l2 vs the jax reference ~3e-3
(tolerance 2e-2).

Math recap (validated numerically):
  w_ch = hdr^3(mask_ch) -> 6 rounds of u' = u - sin(u); centered
  v = u - pi makes each round v' = v + sin(v).  v0 = 2*pi*(x-0.5).
  t = v6_ch0*v6_ch1/pi; vg = r^2(t/pi)... precisely:
  vg1 = t/pi + sin(t/pi), vg2 = vg1 + sin(vg1), m = cm*vg2 + 0.5,
  cm = (1-2*(sx XOR sy))/(2*pi) per area.
  Per axis translations share P: msum = sum(P), q = img*P,
  mean = sum(q)/msum, s2 = sum(((q-mean)*P)^2), var = s2/msum.
  loss = (var4(means) + var4(vars))/2; out = lA/(lA+lB+eps).
"""
import numpy as np
from contextlib import ExitStack
from operator import add as _add

import ml_dtypes

import concourse.bass as bass
import concourse.tile as tile
from concourse import bacc, mybir
from concourse.bass_utils import run_bass_kernel_spmd
import concourse.dve_ops as _dve_ops
from concourse.dve_ops import DveOp
from concourse.dve_spec import Spec, Src0, Src1, C0, C1, C2, Zero, sq

F32 = mybir.dt.float32
BF16 = mybir.dt.bfloat16
AF = mybir.ActivationFunctionType
OP = mybir.AluOpType

N_AREAS = 32768
DIAM = 32
PIX = DIAM * DIAM            # 1024
N_CORES = 8
APC = N_AREAS // N_CORES     # 4096 areas per core
NB = APC // 128              # 32 batches of 128 areas
TWO_PI = float(2.0 * np.pi)
PI = float(np.pi)
EPS = 1e-8

_CACHE = {}
TRACE = False

# ---- engine assignment knobs (A/B tuned on HW) ----
ACT_ROUNDS = (2, 3, 4, 5, 6)  # chain round indices (1..6) run as ACT Sin + DVE add
ACT_POST = True     # vg rounds via ACT Sin + DVE add
SQ_ON_ACT = False   # variance term via STT + ACT Square-accum
PQ_ACT_REDUCE = False  # P/q passes via bf16 TT + ACT Identity-accum
GPS_SUB = ()        # chain rounds whose DVE-add runs on GPSIMD instead
ACT_R1 = False      # round 1 via ACT Sin(scale=2pi) + DVE STT
GPS_TMUL = False    # t = v6_ch0*v6_ch1 on GPSIMD instead of DVE
CM_ONCE = True      # load cm for all batches in one [128, NB] DMA
SKIP_CHAIN = False  # timing decomposition: stats on raw X
SKIP_STATS = False  # timing decomposition: reduce m directly
POOLS = {"x": 2, "u": 3, "s": 2, "t": 6, "m": 2, "img": 2, "pq": 8,
         "tiny": 24}


# ---------------- custom DVE ops ----------------

def _round_p7_ref(in0, in1, s0, s1, imm2):
    x = in0.astype(np.float32)
    w = x * x
    return (x * (s0 + w * (s1 + w * (imm2 + w * in1)))).astype(np.float32)


def _sqmr_ref(in0, in1, s0, s1, imm2):
    b = (((in0.astype(np.float32) + s0) * in1) ** 2).astype(np.float32)
    return b, b.reshape(b.shape[0], -1).sum(axis=-1, keepdims=True)


_w = Src0 * Src0
ROUND_P7_ANT = DveOp(
    "ROUND_P7_ANT",
    Spec(body=Src0 * (C0 + _w * (C1 + _w * (C2 + _w * Src1))),
         reference=_round_p7_ref),
    subdim=False, uops_sha={},
)
SQMR_ANT = DveOp(
    "SQMR_ANT",
    Spec(body=sq((Src0 + C0) * Src1), accum=_add, accum_init=Zero,
         reference=_sqmr_ref),
    subdim=False, uops_sha={},
)


def _register_ops():
    from concourse.dve_uop import DveOpSpec
    from concourse.dve_spec import lower
    for op in (ROUND_P7_ANT, SQMR_ANT):
        if op.name in _dve_ops._SUB_OPCODE_FOR_NAME:
            continue
        for ver in ("v3", "v4"):
            spec = DveOpSpec(name=op.name, opcode=0,
                             uops=lower(op.spec, ver=ver))
            op.uops_sha[ver] = spec.sha(ver)
        row = _dve_ops._CUSTOM_DVE_ROW_BASE + len(_dve_ops.OPS)
        assert row < 0x20
        _dve_ops.OPS.append(op)
        _dve_ops.CUSTOM_DVE_SPECS[op.name] = op.spec
        _dve_ops._SUB_OPCODE_FOR_NAME[op.name] = row


_register_ops()


def fit_round_poly(scale=1.0, npts=20001, w_iters=40):
    """deg-7 odd minimax-ish fit of r(v)=v+sin(v), v = scale*z, as poly in z."""
    th = np.linspace(0, np.pi, npts)
    v = np.pi * (0.5 - 0.5 * np.cos(th))
    y = v + np.sin(v)
    A = np.stack([v ** (2 * k + 1) for k in range(4)], axis=1)
    wts = np.ones_like(v)
    for _ in range(w_iters):
        c, *_ = np.linalg.lstsq(A * wts[:, None], y * wts, rcond=None)
        e = np.abs(A @ c - y)
        wts *= (1.0 + e / (e.max() + 1e-30)) ** 2
        wts /= wts.mean()
    return [float(c[k] * scale ** (2 * k + 1)) for k in range(4)]


C_R1 = fit_round_poly(TWO_PI)   # round 1 from xc = x-0.5
C_RS = fit_round_poly(1.0)      # standard round in v
C_VG = fit_round_poly(1.0 / PI)  # vg1 = r(t/pi) as poly in t


# ---------------- program builder ----------------

def _stats_axis(nc, pools, mAP, m3AP, imgT, img3, axis, S, F):
    """Emit one (mask-half, axis) stats unit.  Writes s1f,s1b,s2f,s2b into
    S[:, 4*axis:...] then F[:, same cols] = S*rm (means/vars)."""
    pq = pools["pq"]
    tiny = pools["tiny"]
    if axis == 0:
        a0, a1 = mAP[:, 64:1024], mAP[:, 0:960]
        i_f, i_b = imgT[:, 64:1024], imgT[:, 0:960]
        shp3 = None
    else:
        a0, a1 = m3AP[:, :, 2:DIAM], m3AP[:, :, 0 : DIAM - 2]
        i_f, i_b = img3[:, :, 2:DIAM], img3[:, :, 0 : DIAM - 2]
        shp3 = True

    def v3(tl):
        return tl[:] if shp3 is None else tl[:].rearrange(
            "p (r c) -> p r c", c=DIAM - 2)

    col = 4 * axis
    P = pq.tile([128, 960], BF16, tag="pq")
    msum = tiny.tile([128, 1], F32, tag="tiny")
    qf = pq.tile([128, 960], BF16, tag="pq")
    qb = pq.tile([128, 960], BF16, tag="pq")
    if PQ_ACT_REDUCE:
        nc.vector.tensor_tensor(v3(P), a0, a1, OP.mult)
        jp = pq.tile([128, 960], BF16, tag="pq")
        nc.scalar.activation(jp[:], P[:], AF.Identity, accum_out=msum[:])
        nc.vector.tensor_tensor(v3(qf), i_f, v3(P), OP.mult)
        jf = pq.tile([128, 960], BF16, tag="pq")
        nc.scalar.activation(jf[:], qf[:], AF.Identity,
                             accum_out=S[:, col : col + 1])
        nc.vector.tensor_tensor(v3(qb), i_b, v3(P), OP.mult)
        jb = pq.tile([128, 960], BF16, tag="pq")
        nc.scalar.activation(jb[:], qb[:], AF.Identity,
                             accum_out=S[:, col + 1 : col + 2])
    else:
        nc.vector.affine_mul_reduce(out=v3(P), accum_out=msum[:], in0=a0,
                                    in1=a1, scale=1.0, bias=0.0)
        nc.vector.affine_mul_reduce(out=v3(qf),
                                    accum_out=S[:, col : col + 1],
                                    in0=i_f, in1=v3(P), scale=1.0, bias=0.0)
        nc.vector.affine_mul_reduce(out=v3(qb),
                                    accum_out=S[:, col + 1 : col + 2],
                                    in0=i_b, in1=v3(P), scale=1.0, bias=0.0)
    rm = tiny.tile([128, 1], F32, tag="tiny")
    nc.vector.reciprocal(rm[:], msum[:])
    nmf = tiny.tile([128, 1], F32, tag="tiny")
    nc.vector.tensor_scalar(nmf[:], S[:, col : col + 1], rm[:], -1.0,
                            OP.mult, OP.mult)
    nmb = tiny.tile([128, 1], F32, tag="tiny")
    nc.vector.tensor_scalar(nmb[:], S[:, col + 1 : col + 2], rm[:], -1.0,
                            OP.mult, OP.mult)
    if SQ_ON_ACT:
        df = pq.tile([128, 960], BF16, tag="pq")
        nc.vector.scalar_tensor_tensor(df[:], qf[:], nmf[:], P[:],
                                       OP.add, OP.mult)
        junk = pq.tile([128, 960], BF16, tag="pq")
        nc.scalar.activation(junk[:], df[:], AF.Square, scale=1.0,
                             accum_out=S[:, col + 2 : col + 3])
        db = pq.tile([128, 960], BF16, tag="pq")
        nc.vector.scalar_tensor_tensor(db[:], qb[:], nmb[:], P[:],
                                       OP.add, OP.mult)
        junk2 = pq.tile([128, 960], BF16, tag="pq")
        nc.scalar.activation(junk2[:], db[:], AF.Square, scale=1.0,
                             accum_out=S[:, col + 3 : col + 4])
    else:
        sf = pq.tile([128, 960], BF16, tag="pq")
        nc.vector._custom_dve(SQMR_ANT, out=sf[:], in0=qf[:], in1=P[:],
                              s0=nmf[:], s1=0.0,
                              accum_out=S[:, col + 2 : col + 3])
        sb = pq.tile([128, 960], BF16, tag="pq")
        nc.vector._custom_dve(SQMR_ANT, out=sb[:], in0=qb[:], in1=P[:],
                              s0=nmb[:], s1=0.0,
                              accum_out=S[:, col + 3 : col + 4])
    nc.vector.tensor_scalar(F[:, col : col + 4], S[:, col : col + 4], rm[:],
                            None, OP.mult)


def build_body2(nc, drams, nbatches, full):
    xA_d, xB_d, img_d, cm_d, out_d = drams
    with tile.TileContext(nc) as tc, ExitStack() as ctx:
        pools = {
            k: ctx.enter_context(tc.tile_pool(name=k, bufs=POOLS[k]))
            for k in ("x", "u", "s", "t", "m", "img", "pq", "tiny")
        }
        pools.update({
            "const": ctx.enter_context(tc.tile_pool(name="const", bufs=1)),
            "acc": ctx.enter_context(tc.tile_pool(name="acc", bufs=1)),
            "fin": ctx.enter_context(tc.tile_pool(name="fin", bufs=1)),
        })
        co = pools["const"]
        c3_r1 = co.tile([128, 4096], F32, tag="c3r1")
        nc.vector.memset(c3_r1[:], C_R1[3])
        c3_rs = co.tile([128, 4096], F32, tag="c3rs")
        nc.vector.memset(c3_rs[:], C_RS[3])
        c3_vg = co.tile([128, 2048], F32, tag="c3vg")
        nc.vector.memset(c3_vg[:], C_VG[3])

        # per-batch sum tiles: [128, nrot, 2 masks, 8].  In bench mode
        # (full=False) rotate through a 32-batch window so SBUF stays fixed.
        nrot = min(nbatches, 32)
        Sall = pools["acc"].tile([128, nrot * 16], F32, tag="sall")
        Fall = pools["acc"].tile([128, nrot * 16], F32, tag="fall")

        if CM_ONCE:
            cm_all = pools["const"].tile([128, nbatches], F32, tag="cmall")
            nc.sync.dma_start(cm_all[:], cm_d.ap()[:, :])

        for b in range(nbatches):
            r0 = b * 128 if full else 0
            X = pools["x"].tile([128, 4096], BF16, tag="x")
            nc.sync.dma_start(X[:, 0:2048], xA_d.ap()[r0 : r0 + 128, :])
            nc.sync.dma_start(X[:, 2048:4096], xB_d.ap()[r0 : r0 + 128, :])
            img = pools["img"].tile([128, PIX], BF16, tag="img")
            nc.sync.dma_start(img[:], img_d.ap()[r0 : r0 + 128, :])
            if CM_ONCE:
                cm = cm_all[:, b : b + 1]
            else:
                cmt = pools["tiny"].tile([128, 1], F32, tag="cm")
                nc.sync.dma_start(cmt[:], cm_d.ap()[r0 : r0 + 128, :])
                cm = cmt[:]

            # chain: 6 rounds
            u = X
            for r in range(1, 7) if not SKIP_CHAIN else ():
                cs = C_R1 if r == 1 else C_RS
                c3t = c3_r1 if r == 1 else c3_rs
                if r == 1 and ACT_R1:
                    s = pools["s"].tile([128, 4096], BF16, tag="s")
                    nc.scalar.activation(s[:], u[:], AF.Sin, scale=TWO_PI)
                    u2 = pools["u"].tile([128, 4096], BF16, tag="u")
                    nc.vector.scalar_tensor_tensor(u2[:], u[:], TWO_PI, s[:],
                                                   OP.mult, OP.add)
                    u = u2
                    continue
                if r in ACT_ROUNDS:
                    s = pools["s"].tile([128, 4096], BF16, tag="s")
                    if r == 1:
                        # v0 = 2*pi*xc ; s = sin(v0); u1 = v0 + s needs v0:
                        # do it as one fused round op instead (cheaper)
                        u2 = pools["u"].tile([128, 4096], BF16, tag="u")
                        nc.vector._custom_dve(ROUND_P7_ANT, out=u2[:],
                                              in0=u[:], in1=c3t[:],
                                              s0=cs[0], s1=cs[1], imm2=cs[2])
                        u = u2
                        continue
                    nc.scalar.activation(s[:], u[:], AF.Sin, scale=1.0)
                    u2 = pools["u"].tile([128, 4096], BF16, tag="u")
                    eng = nc.gpsimd if r in GPS_SUB else nc.vector
                    eng.tensor_tensor(u2[:], u[:], s[:], OP.add)
                    u = u2
                else:
                    u2 = pools["u"].tile([128, 4096], BF16, tag="u")
                    nc.vector._custom_dve(ROUND_P7_ANT, out=u2[:], in0=u[:],
                                          in1=c3t[:], s0=cs[0], s1=cs[1],
                                          imm2=cs[2])
                    u = u2

            if SKIP_CHAIN:
                m = u  # raw xc values as a stand-in mask (timing decomp)
                img3 = img[:].rearrange("p (r c) -> p r c", c=DIAM)
                for h in range(2):
                    mh = m[:, h * PIX : (h + 1) * PIX]
                    m3 = mh.rearrange("p (r c) -> p r c", c=DIAM)
                    br = b % nrot
                    S = Sall[:, (br * 2 + h) * 8 : (br * 2 + h) * 8 + 8]
                    F = Fall[:, (br * 2 + h) * 8 : (br * 2 + h) * 8 + 8]
                    _stats_axis(nc, pools, mh, m3, img[:], img3, 0, S, F)
                    _stats_axis(nc, pools, mh, m3, img[:], img3, 1, S, F)
                continue

            # t = v6_ch0 * v6_ch1 (channels interleaved within each mask half)
            uv = u[:].rearrange("p (m c two) -> p m c two", m=2, two=2)
            t = pools["t"].tile([128, 2048], BF16, tag="t")
            tv = t[:].rearrange("p (m c) -> p m c", m=2)
            teng = nc.gpsimd if GPS_TMUL else nc.vector
            teng.tensor_tensor(tv, uv[:, :, :, 0], uv[:, :, :, 1],
                               OP.mult)
            # vg1 = r(t/pi) (poly in t), vg2 = r(vg1)
            if ACT_POST:
                s1t = pools["t"].tile([128, 2048], BF16, tag="t")
                nc.scalar.activation(s1t[:], t[:], AF.Sin, scale=1.0 / PI)
                vg1 = pools["t"].tile([128, 2048], BF16, tag="t")
                nc.vector.scalar_tensor_tensor(vg1[:], t[:], 1.0 / PI,
                                               s1t[:], OP.mult, OP.add)
                s2t = pools["t"].tile([128, 2048], BF16, tag="t")
                nc.scalar.activation(s2t[:], vg1[:], AF.Sin, scale=1.0)
                vg2 = pools["t"].tile([128, 2048], BF16, tag="t")
                nc.vector.tensor_tensor(vg2[:], vg1[:], s2t[:], OP.add)
            else:
                vg1 = pools["t"].tile([128, 2048], BF16, tag="t")
                nc.vector._custom_dve(ROUND_P7_ANT, out=vg1[:], in0=t[:],
                                      in1=c3_vg[:], s0=C_VG[0], s1=C_VG[1],
                                      imm2=C_VG[2])
                vg2 = pools["t"].tile([128, 2048], BF16, tag="t")
                nc.vector._custom_dve(ROUND_P7_ANT, out=vg2[:], in0=vg1[:],
                                      in1=c3_rs[:, 0:2048], s0=C_RS[0],
                                      s1=C_RS[1], imm2=C_RS[2])
            # m = cm*vg2 + 0.5
            m = pools["m"].tile([128, 2048], BF16, tag="m")
            nc.vector.tensor_scalar(m[:], vg2[:], cm, 0.5, OP.mult, OP.add)

            img3 = img[:].rearrange("p (r c) -> p r c", c=DIAM)
            for h in range(2):
                mh = m[:, h * PIX : (h + 1) * PIX]
                br = b % nrot
                S = Sall[:, (br * 2 + h) * 8 : (br * 2 + h) * 8 + 8]
                F = Fall[:, (br * 2 + h) * 8 : (br * 2 + h) * 8 + 8]
                if SKIP_STATS:
                    nc.vector.tensor_reduce(F[:, 0:1], mh,
                                            mybir.AxisListType.X, OP.add)
                    continue
                m3 = mh.rearrange("p (r c) -> p r c", c=DIAM)
                _stats_axis(nc, pools, mh, m3, img[:], img3, 0, S, F)
                _stats_axis(nc, pools, mh, m3, img[:], img3, 1, S, F)

        # ---- finish phase (vectorized over batches) ----
        fin = pools["fin"]
        nb = nrot
        # sums over the 4 translations: view [128, u, (ax 2), (mv 2), (fb 2)]
        # F layout per (u, ax): [meanf, meanb, varf, varb]
        FS = fin.tile([128, nb * 2 * 2], F32, tag="FS")   # [u, mv, 1]
        Fsum_view = Fall[:].rearrange("p (u ax mv f) -> p u mv ax f",
                                      ax=2, mv=2, f=2)
        FSv = FS[:].rearrange("p (u mv) -> p u mv", mv=2)
        nc.vector.tensor_reduce(FSv, Fsum_view, mybir.AxisListType.XY, OP.add)
        F2 = fin.tile([128, nb * 16], F32, tag="F2")
        nc.vector.tensor_tensor(F2[:], Fall[:], Fall[:], OP.mult)
        F2S = fin.tile([128, nb * 2 * 2], F32, tag="F2S")
        F2sum_view = F2[:].rearrange("p (u ax mv f) -> p u mv ax f",
                                     ax=2, mv=2, f=2)
        F2Sv = F2S[:].rearrange("p (u mv) -> p u mv", mv=2)
        nc.vector.tensor_reduce(F2Sv, F2sum_view, mybir.AxisListType.XY,
                                OP.add)
        # scaled loss per (u): L = sum_mv (4*S2 - S^2)  (= 32 * loss)
        SS = fin.tile([128, nb * 2 * 2], F32, tag="SS")
        nc.vector.tensor_tensor(SS[:], FS[:], FS[:], OP.mult)
        Lmv = fin.tile([128, nb * 2 * 2], F32, tag="Lmv")
        nc.vector.scalar_tensor_tensor(Lmv[:], F2S[:], 4.0, SS[:],
                                       OP.mult, OP.subtract)
        L = fin.tile([128, nb * 2], F32, tag="L")
        Lv = L[:].rearrange("p (u one) -> p u one", one=1)
        nc.vector.tensor_reduce(Lv, Lmv[:].rearrange(
            "p (u mv) -> p u mv", mv=2), mybir.AxisListType.X, OP.add)
        # out = LA / (LA + LB + 32*eps)
        Lm = L[:].rearrange("p (b two) -> p b two", two=2)
        den = fin.tile([128, nb], F32, tag="den")
        nc.vector.tensor_tensor(den[:], Lm[:, :, 0], Lm[:, :, 1], OP.add)
        den2 = fin.tile([128, nb], F32, tag="den2")
        nc.vector.tensor_scalar(den2[:], den[:], 32.0 * EPS, None, OP.add)
        rden = fin.tile([128, nb], F32, tag="rden")
        nc.vector.reciprocal(rden[:], den2[:])
        outc = fin.tile([128, nb], F32, tag="outc")
        nc.vector.tensor_tensor(outc[:], Lm[:, :, 0], rden[:], OP.mult)
        nc.sync.dma_start(out_d.ap()[:, 0:nb], outc[:])


def build_nc2(nbatches=NB, full=True):
    nc = bacc.Bacc("TRN2", target_bir_lowering=False, debug=False,
                   num_devices=N_CORES)
    rows = nbatches * 128 if full else 128
    xA_d = nc.dram_tensor("xA", [rows, 2 * PIX], BF16, kind="ExternalInput")
    xB_d = nc.dram_tensor("xB", [rows, 2 * PIX], BF16, kind="ExternalInput")
    img_d = nc.dram_tensor("img", [rows, PIX], BF16, kind="ExternalInput")
    cm_shape = [128, nbatches] if CM_ONCE else [rows, 1]
    cm_d = nc.dram_tensor("cm", cm_shape, F32, kind="ExternalInput")
    out_d = nc.dram_tensor("out", [128, nbatches], F32,
                           kind="ExternalOutput")
    build_body2(nc, (xA_d, xB_d, img_d, cm_d, out_d), nbatches, full)
    nc.finalize()
    return nc


def _prep(resized_image, mask_combined, mask_combined_alt, mask_index):
    bf = ml_dtypes.bfloat16
    xA = (np.asarray(mask_combined, np.float32).reshape(N_AREAS, 2 * PIX)
          - np.float32(0.5)).astype(bf)
    xB = (np.asarray(mask_combined_alt, np.float32).reshape(N_AREAS, 2 * PIX)
          - np.float32(0.5)).astype(bf)
    img = np.asarray(resized_image, np.float32).reshape(
        N_AREAS, PIX).astype(bf)
    idx = np.asarray(mask_index).astype(np.int64)
    p = ((idx % 2) ^ (idx // 2)).astype(np.float32)
    cm = ((1.0 - 2.0 * p) / (2.0 * np.pi)).astype(np.float32)
    if CM_ONCE:
        # per-core [128, NB]: area a = b*128 + p within the core slice
        cm = np.ascontiguousarray(
            cm.reshape(N_CORES, NB, 128).transpose(0, 2, 1))
    else:
        cm = cm.reshape(N_AREAS, 1)
    return xA, xB, img, cm


def kernel(resized_image, mask_combined, mask_combined_alt, mask_index):
    xA, xB, img, cm = _prep(resized_image, mask_combined,
                            mask_combined_alt, mask_index)
    if "nc" not in _CACHE:
        _CACHE["nc"] = build_nc2()
    nc = _CACHE["nc"]
    in_maps = []
    for c in range(N_CORES):
        s = slice(c * APC, (c + 1) * APC)
        in_maps.append({"xA": xA[s], "xB": xB[s], "img": img[s],
                        "cm": cm[c] if CM_ONCE else cm[s]})
    res = run_bass_kernel_spmd(nc, in_maps, core_ids=list(range(N_CORES)),
                               trace=TRACE)
    outs = []
    for c in range(N_CORES):
        o = res.results[c]["out"]          # [128, NB]
        outs.append(np.ascontiguousarray(o.T).reshape(APC))
    return np.concatenate(outs).astype(np.float32)



# revision 8
# speedup vs baseline: 1.7010x; 1.7010x over previous
"""Trainium2 Bass kernel v4 for nn_Apply_on_single_area (segment_reduce).

Self-contained: accepts FULL inputs, shards areas across 8 NeuronCores
(pure data parallel), returns FULL [32768] f32 output.

Key ideas vs the v2 baseline (2.01 ms):
 - The 6-round per-channel smoothing chain collapses to ONE ACT Tanh.
   In centered v-units (v = 2pi*(x-0.5)) each round r(v) = v + sin(v)
   has slope 2 at 0 and r'(+-pi) = 0.  The 6-round composite R6 has
   slope 2^6 and saturates at +-pi; pi*tanh(64/pi * v) matches the
   transition core exactly (same slope) and its tail mismatch is
   quadratically squashed by the two exact post-combine rounds
   (super-attracting fixed points).  End-to-end l2 vs the jax
   reference: 3.2e-4 fp32 / 2.1e-3 bf16 (tolerance 2e-2).
 - The per-area sign sigma = (2sx-1)(2sy-1) from mask_index is folded
   into the sign of channel-0's centered mask on the host (the combine
   is odd), so the device kernel needs no per-area constants.
 - Engine balance: DVE keeps only the fused multiply-reduces (1x ops)
   and cheap 2x/4x passes; the mean-subtract passes run on GPSIMD; the
   variance squares accumulate on ACT (Square); tanh/sin/square are
   pinned to one ACT table set (silu_and_others) to avoid ~2.7us
   table reloads between batches.

Pipeline per batch of 128 areas (bf16 bodies, fp32 accums):
  ACT: H = tanh(128 * xc)                      [128, 4096]
  DVE: tau = H_ch0 * H_ch1                     [128, 2048]
       taupi = pi * tau            (4x tensor_scalar)
  ACT: S1 = sin(pi * tau)
  DVE: T1 = taupi + S1             (exact round 1)
  ACT: S2 = sin(T1)
  DVE: U = T1 + S2                 (exact round 2)
       M = U/(2pi) + 0.5
  stats per (mask half, axis), exact reference formulas:
    P = M * M_shift   (affine_mul_reduce -> body + msum)
    qf/qb = img_f/b * P  (affine_mul_reduce -> bodies + sums)
    rm = 1/msum (batched recip); mean = sum(q) * rm (4x ts)
    df = (q - mean) * P   [GPSIMD scalar_tensor_tensor]
    s2 = sum(df^2)        [ACT Square accumulate]
  finish phase (vectorized over batches): F = S*rm, means/vars ->
  translation variances -> loss ratio.
"""
import numpy as np
from contextlib import ExitStack

import ml_dtypes

import concourse.bass as bass
import concourse.tile as tile
from concourse import bacc, mybir
from concourse import hw_specs as _hw_specs
from concourse.bass_utils import run_bass_kernel_spmd

F32 = mybir.dt.float32
BF16 = mybir.dt.bfloat16
AF = mybir.ActivationFunctionType
OP = mybir.AluOpType

N_AREAS = 32768
DIAM = 32
PIX = DIAM * DIAM            # 1024
N_CORES = 8
APC = N_AREAS // N_CORES     # 4096 areas per core
NB = APC // 128              # 32 batches of 128 areas
TWO_PI = float(2.0 * np.pi)
PI = float(np.pi)
EPS = 1e-8
TANH_SCALE = 128.0           # = 2 * 2^6: slope of R6 in centered-x units

_CACHE = {}
TRACE = False

# ---- engine assignment knobs ----
SQ_ON_ACT = True    # variance squares via ACT Square-accum (else DVE AMR)
# df = (q - mean) * P construction:
#  "gps": DVE 4x tensor_scalar subtract + GPSIMD tensor_tensor multiply
#  "stt": single DVE scalar_tensor_tensor (1x)
DF_MODE = "gps"
DF_GPS_COUNT = 8    # how many of the 8 df passes use the gps path
GPS_TAU = False     # tau product on GPSIMD
GPS_TAUPI = True    # taupi scale on GPSIMD
GPS_U = False       # round-2 add on GPSIMD
GPS_M = True        # m affine on GPSIMD
POOLS = {"x": 3, "h": 3, "t": 9, "m": 4, "img": 4, "pq": 32, "tiny": 24}


def _pin_act_tables():
    """Keep Tanh/Sin/Square resolvable only via silu_and_others so the
    table-load chooser never alternates sets between tanh and sin calls.
    Set ids (dict order) are preserved; other sets just lose the three
    functions we use, which nothing else in this kernel calls."""
    if getattr(_hw_specs, "_ant_pinned_tables", False):
        return
    orig = _hw_specs.get_activation_tables
    pin = {AF.Tanh, AF.Sin, AF.Square}

    def patched(arch):
        t = orig(arch)
        home = t.get("silu_and_others")
        if home and pin <= home:
            t = {k: (v if k == "silu_and_others" else v - pin)
                 for k, v in t.items()}
        return t

    _hw_specs.get_activation_tables = patched
    bacc.get_activation_tables = patched
    _hw_specs._ant_pinned_tables = True


_pin_act_tables()


def _stats_axis(nc, pools, mAP, m3AP, imgT, img3, axis, S, msum_col):
    """Emit the multiply-reduce part of one (mask-half, axis) unit.
    Writes sum_qf, sum_qb into S cols [4*axis, 4*axis+1], msum into
    msum_col, and returns (P, qf, qb) tiles for the variance stage."""
    pq = pools["pq"]
    if axis == 0:
        a0, a1 = mAP[:, 64:1024], mAP[:, 0:960]
        i_f, i_b = imgT[:, 64:1024], imgT[:, 0:960]
        shp3 = None
    else:
        a0, a1 = m3AP[:, :, 2:DIAM], m3AP[:, :, 0 : DIAM - 2]
        i_f, i_b = img3[:, :, 2:DIAM], img3[:, :, 0 : DIAM - 2]
        shp3 = True

    def v3(tl):
        return tl[:] if shp3 is None else tl[:].rearrange(
            "p (r c) -> p r c", c=DIAM - 2)

    col = 4 * axis
    P = pq.tile([128, 960], BF16, tag="pq")
    qf = pq.tile([128, 960], BF16, tag="pq")
    qb = pq.tile([128, 960], BF16, tag="pq")
    nc.vector.affine_mul_reduce(out=v3(P), accum_out=msum_col, in0=a0,
                                in1=a1, scale=1.0, bias=0.0)
    nc.vector.affine_mul_reduce(out=v3(qf), accum_out=S[:, col : col + 1],
                                in0=i_f, in1=v3(P), scale=1.0, bias=0.0)
    nc.vector.affine_mul_reduce(out=v3(qb),
                                accum_out=S[:, col + 1 : col + 2],
                                in0=i_b, in1=v3(P), scale=1.0, bias=0.0)
    return P, qf, qb


def _variance_stage(nc, pools, units, rm4, S_of):
    """Per batch: means from the accumulated sums, then df = (q-mean)*P on
    GPSIMD and Square-accumulate on ACT."""
    pq = pools["pq"]
    tiny = pools["tiny"]
    for k, (h, axis, P, qf, qb) in enumerate(units):
        S = S_of(h)
        col = 4 * axis
        rmc = rm4[:, 2 * h + axis : 2 * h + axis + 1]
        for d, q in ((0, qf), (1, qb)):
            mean = tiny.tile([128, 1], F32, tag="tiny")
            nc.vector.tensor_scalar(mean[:], S[:, col + d : col + d + 1],
                                    rmc, None, OP.mult)
            df = pq.tile([128, 960], BF16, tag="pq")
            if DF_MODE == "gps" and 2 * k + d < DF_GPS_COUNT:
                qs = pq.tile([128, 960], BF16, tag="pq")
                nc.vector.tensor_scalar(qs[:], q[:], mean[:], None,
                                        OP.subtract)
                nc.gpsimd.tensor_tensor(df[:], qs[:], P[:], OP.mult)
            else:
                nc.vector.scalar_tensor_tensor(df[:], q[:], mean[:], P[:],
                                               OP.subtract, OP.mult)
            if SQ_ON_ACT:
                junk = pq.tile([128, 960], BF16, tag="pq")
                nc.scalar.activation(junk[:], df[:], AF.Square, scale=1.0,
                                     accum_out=S[:, col + 2 + d : col + 3 + d])
            else:
                nc.vector.affine_mul_reduce(
                    out=df[:], accum_out=S[:, col + 2 + d : col + 3 + d],
                    in0=df[:], in1=df[:], scale=1.0, bias=0.0)


def build_body4(nc, drams, nbatches, full):
    xA_d, xB_d, img_d, out_d = drams
    with tile.TileContext(nc) as tc, ExitStack() as ctx:
        pools = {
            k: ctx.enter_context(tc.tile_pool(name=k, bufs=POOLS[k]))
            for k in ("x", "h", "t", "m", "img", "pq", "tiny")
        }
        pools.update({
            "acc": ctx.enter_context(tc.tile_pool(name="acc", bufs=1)),
            "fin": ctx.enter_context(tc.tile_pool(name="fin", bufs=1)),
        })

        nrot = min(nbatches, 32)
        Sall = pools["acc"].tile([128, nrot * 16], F32, tag="sall")
        Rall = pools["acc"].tile([128, nrot * 4], F32, tag="rall")

        for b in range(nbatches):
            r0 = b * 128 if full else 0
            X = pools["x"].tile([128, 4096], BF16, tag="x")
            nc.sync.dma_start(X[:, 0:2048], xA_d.ap()[r0 : r0 + 128, :])
            nc.sync.dma_start(X[:, 2048:4096], xB_d.ap()[r0 : r0 + 128, :])
            img = pools["img"].tile([128, PIX], BF16, tag="img")
            nc.sync.dma_start(img[:], img_d.ap()[r0 : r0 + 128, :])

            # chain: H = tanh(128*xc); two exact rounds on pi*(H0*H1); M
            H = pools["h"].tile([128, 4096], BF16, tag="h")
            nc.scalar.activation(H[:], X[:], AF.Tanh, scale=TANH_SCALE)
            Hv = H[:].rearrange("p (m c two) -> p m c two", m=2, two=2)
            tau = pools["t"].tile([128, 2048], BF16, tag="t")
            tauv = tau[:].rearrange("p (m c) -> p m c", m=2)
            teng = nc.gpsimd if GPS_TAU else nc.vector
            teng.tensor_tensor(tauv, Hv[:, :, :, 0], Hv[:, :, :, 1], OP.mult)
            taupi = pools["t"].tile([128, 2048], BF16, tag="t")
            tpeng = nc.gpsimd if GPS_TAUPI else nc.vector
            tpeng.tensor_scalar(taupi[:], tau[:], PI, None, OP.mult)
            S1 = pools["t"].tile([128, 2048], BF16, tag="t")
            nc.scalar.activation(S1[:], tau[:], AF.Sin, scale=PI)
            T1 = pools["t"].tile([128, 2048], BF16, tag="t")
            nc.vector.tensor_tensor(T1[:], taupi[:], S1[:], OP.add)
            S2 = pools["t"].tile([128, 2048], BF16, tag="t")
            nc.scalar.activation(S2[:], T1[:], AF.Sin, scale=1.0)
            U = pools["t"].tile([128, 2048], BF16, tag="t")
            ueng = nc.gpsimd if GPS_U else nc.vector
            ueng.tensor_tensor(U[:], T1[:], S2[:], OP.add)
            M = pools["m"].tile([128, 2048], BF16, tag="m")
            meng = nc.gpsimd if GPS_M else nc.vector
            meng.tensor_scalar(M[:], U[:], 1.0 / TWO_PI, 0.5, OP.mult,
                               OP.add)

            img3 = img[:].rearrange("p (r c) -> p r c", c=DIAM)
            br = b % nrot

            def S_of(h):
                return Sall[:, (br * 2 + h) * 8 : (br * 2 + h) * 8 + 8]

            msum4 = pools["tiny"].tile([128, 4], F32, tag="tiny4")
            units = []
            for h in range(2):
                mh = M[:, h * PIX : (h + 1) * PIX]
                m3 = mh.rearrange("p (r c) -> p r c", c=DIAM)
                for axis in (0, 1):
                    P, qf, qb = _stats_axis(
                        nc, pools, mh, m3, img[:], img3, axis, S_of(h),
                        msum4[:, 2 * h + axis : 2 * h + axis + 1])
                    units.append((h, axis, P, qf, qb))
            rm4 = Rall[:, br * 4 : br * 4 + 4]
            nc.vector.reciprocal(rm4, msum4[:])
            _variance_stage(nc, pools, units, rm4, S_of)

        # ---- finish phase (vectorized over batches) ----
        fin = pools["fin"]
        nb = nrot
        # F = S * rm  (rm broadcast over the 4 cols of each unit)
        Fall = fin.tile([128, nb * 16], F32, tag="fall")
        Sv = Sall[:].rearrange("p (u f) -> p u f", f=4)     # u = (br, h, ax)
        Fv = Fall[:].rearrange("p (u f) -> p u f", f=4)
        Rv = Rall[:].rearrange("p (u one) -> p u one", one=1).broadcast_to(
            (128, nb * 4, 4))
        nc.vector.tensor_tensor(Fv, Sv, Rv, OP.mult)
        FS = fin.tile([128, nb * 2 * 2], F32, tag="FS")
        Fsum_view = Fall[:].rearrange("p (u ax mv f) -> p u mv ax f",
                                      ax=2, mv=2, f=2)
        FSv = FS[:].rearrange("p (u mv) -> p u mv", mv=2)
        nc.vector.tensor_reduce(FSv, Fsum_view, mybir.AxisListType.XY, OP.add)
        F2 = fin.tile([128, nb * 16], F32, tag="F2")
        nc.vector.tensor_tensor(F2[:], Fall[:], Fall[:], OP.mult)
        F2S = fin.tile([128, nb * 2 * 2], F32, tag="F2S")
        F2sum_view = F2[:].rearrange("p (u ax mv f) -> p u mv ax f",
                                     ax=2, mv=2, f=2)
        F2Sv = F2S[:].rearrange("p (u mv) -> p u mv", mv=2)
        nc.vector.tensor_reduce(F2Sv, F2sum_view, mybir.AxisListType.XY,
                                OP.add)
        SS = fin.tile([128, nb * 2 * 2], F32, tag="SS")
        nc.vector.tensor_tensor(SS[:], FS[:], FS[:], OP.mult)
        Lmv = fin.tile([128, nb * 2 * 2], F32, tag="Lmv")
        nc.vector.scalar_tensor_tensor(Lmv[:], F2S[:], 4.0, SS[:],
                                       OP.mult, OP.subtract)
        L = fin.tile([128, nb * 2], F32, tag="L")
        Lv = L[:].rearrange("p (u one) -> p u one", one=1)
        nc.vector.tensor_reduce(Lv, Lmv[:].rearrange(
            "p (u mv) -> p u mv", mv=2), mybir.AxisListType.X, OP.add)
        Lm = L[:].rearrange("p (b two) -> p b two", two=2)
        den = fin.tile([128, nb], F32, tag="den")
        nc.vector.tensor_tensor(den[:], Lm[:, :, 0], Lm[:, :, 1], OP.add)
        den2 = fin.tile([128, nb], F32, tag="den2")
        nc.vector.tensor_scalar(den2[:], den[:], 32.0 * EPS, None, OP.add)
        rden = fin.tile([128, nb], F32, tag="rden")
        nc.vector.reciprocal(rden[:], den2[:])
        outc = fin.tile([128, nb], F32, tag="outc")
        nc.vector.tensor_tensor(outc[:], Lm[:, :, 0], rden[:], OP.mult)
        nc.sync.dma_start(out_d.ap()[:, 0:nb], outc[:])


def build_nc2(nbatches=NB, full=True):
    nc = bacc.Bacc("TRN2", target_bir_lowering=False, debug=False,
                   num_devices=N_CORES)
    rows = nbatches * 128 if full else 128
    xA_d = nc.dram_tensor("xA", [rows, 2 * PIX], BF16, kind="ExternalInput")
    xB_d = nc.dram_tensor("xB", [rows, 2 * PIX], BF16, kind="ExternalInput")
    img_d = nc.dram_tensor("img", [rows, PIX], BF16, kind="ExternalInput")
    out_d = nc.dram_tensor("out", [128, nbatches], F32,
                           kind="ExternalOutput")
    build_body4(nc, (xA_d, xB_d, img_d, out_d), nbatches, full)
    nc.finalize()
    return nc


def _prep(resized_image, mask_combined, mask_combined_alt, mask_index):
    bf = ml_dtypes.bfloat16
    idx = np.asarray(mask_index).astype(np.int64)
    sig = (1.0 - 2.0 * ((idx % 2) ^ (idx // 2))).astype(np.float32)

    def center(m):
        x = np.asarray(m, np.float32).reshape(N_AREAS, PIX, 2) - np.float32(
            0.5)
        x[:, :, 0] *= sig[:, None]
        return x.reshape(N_AREAS, 2 * PIX).astype(bf)

    xA = center(mask_combined)
    xB = center(mask_combined_alt)
    img = np.asarray(resized_image, np.float32).reshape(
        N_AREAS, PIX).astype(bf)
    return xA, xB, img


def kernel(resized_image, mask_combined, mask_combined_alt, mask_index):
    xA, xB, img = _prep(resized_image, mask_combined,
                        mask_combined_alt, mask_index)
    if "nc" not in _CACHE:
        _CACHE["nc"] = build_nc2()
    nc = _CACHE["nc"]
    in_maps = []
    for c in range(N_CORES):
        s = slice(c * APC, (c + 1) * APC)
        in_maps.append({"xA": xA[s], "xB": xB[s], "img": img[s]})
    res = run_bass_kernel_spmd(nc, in_maps, core_ids=list(range(N_CORES)),
                               trace=TRACE)
    outs = []
    for c in range(N_CORES):
        o = res.results[c]["out"]          # [128, NB]
        outs.append(np.ascontiguousarray(o.T).reshape(APC))
    return np.concatenate(outs).astype(np.float32)
